# revision 32
# baseline (speedup 1.0000x reference)
"""CARAFE kernel for 8 TRN2 NeuronCores (Bass/Tile, SPMD).

Math (see reference):
  k0   = w_comp @ x + b_comp                 (64, 32, 32)      1x1 conv
  kc   = w_ker (*) k0 + b_ker                (102400, 32, 32)  3x3 conv, pad 1
  k    = softmax(kc.reshape(4, 25600, H, W), axis=1)
  ksum = k.sum(axis=1)                       (4, 32, 32)       == S/S (==1+eps)
  out  = (x[:, :, None] * ksum[:, None]).reshape(1, 256, 64, 64)

Sharding: core c = (g, h) with g = c//2 (softmax/scale group), h = c%2
(pixel half: image rows 16h..16h+16).  Each core computes its group's
FULL 25600 conv channels for its OWN 512 pixels, so the softmax group
sum S is core-local -- no collective at all.

v2 layout (vs the 161.5us baseline, which was ScalarE-bound at
1295ns/pair-iteration: 1114ns EXP[128,1024] + 181ns accumulator
readout, with the PE stretched to match):

  * Channel tiles are processed in GROUPS of 4 (2 psum banks per 512
    tile -> [128, 2048] psum spanning 4 banks, 2 groups in flight =
    all 8 banks).  One 2048-wide Exp eviction per group amortizes the
    ScalarE per-instruction overhead (~300cy) 2x better, and only a
    subset of groups pay the 181ns ACTIVATION_READ_ACCUMULATOR --
    the rest get their per-pixel partial sums from a DVE
    tensor_reduce of the (otherwise-dead) exp output tile, keeping
    the otherwise-idle Vector engine busy instead of ScalarE.
  * tap-8 tails run as concurrent row-tiled DR matmuls, 32 array rows
    at base partitions 0 and 64 (offset 96 is the unusable PE quadrant
    3, so 4-way packing is not possible): two tail slots per 4-tile
    group, each slot covering 2 tiles concurrently with K=64 via
    DoubleRow on 32 partitions.
  * The im2col trick is unchanged: the compressed image (18 rows incl
    halo) lives in 23-row x 32-col zero-framed fp8 strips; each tap is
    a flat-shifted SBUF->SBUF DMA copy (shift (dh-1)*32+(dw-1)), with
    row-crossing leak columns zeroed by tiny DVE memsets.  The tail
    strip V8DR holds tap 8 in 4-way row-tiled DR layout: partition
    32t+q, slot i = image channel q+32i (each 32-row block is a plain
    partition-offset copy of half the base strip).
  * Startup: the compress-conv input xf is split across two DGE
    queues (sync + pool) so the 294KB load halves; weight prefetch is
    issued on the sync queue strictly after the strip copies so the
    copy DMAs (which gate the first matmul) are never stuck behind
    512KB of weights (baseline lesson).
  * Finals are per-mt pipelined; spart is 4 per-mt tiles so each
    m-tile's S reduce only waits on its own last eviction; the 4
    output stores go out on 4 different engine DGEs.
  * Weights are scaled x16 on the host before fp8e4 quantization
    (their 0.05 sigma sits in e4m3's subnormal range); the Exp
    eviction's activation scale of 1/16 undoes it exactly.  Softmax
    sums are divided by themselves (ksum == 1 in exact arithmetic), so
    conv precision does not reach the output.  A separate exact
    variant (the v1 kernel, K=65 ones row + halo mask) is built
    lazily if nonzero biases are ever passed.
"""

import numpy as np

import concourse.bass as bass
import concourse.mybir as mybir
import concourse.tile as tile
from concourse import bacc
from concourse.bass_utils import run_bass_kernel_spmd

F32 = mybir.dt.float32
BF16 = mybir.dt.bfloat16
FP8 = mybir.dt.float8e4
AF = mybir.ActivationFunctionType
DR = mybir.MatmulPerfMode.DoubleRow

# Problem constants
C, H, W = 256, 32, 32
CH = 64                    # compressed channels
SC = 2                     # upsample scale
OC_TOTAL = 102400
NCORES = 8
GCH = OC_TOTAL // 4        # 25600 channels per softmax group (= per core)
NT = GCH // 512            # 50 channel tiles of 512
NG = 13                    # channel-tile groups per m-tile: 12x4 + 1x2
GT = [4] * 12 + [2]        # tiles per group
PIX = 512                  # pixels per core (16 image rows)
MT = PIX // 128            # 4 pixel tiles of 128 (4 image rows each)
HLOC = 18                  # local k0 rows incl 1-row halo each side
NLOC = HLOC * W            # 576 compress-conv pixels
FROWS = 23                 # zero-framed strip rows
FRAME = FROWS * W          # 736 bytes per image copy (div by 16)
WSCALE = 16.0              # host weight scale, undone by Exp's 1/16

# groups whose partial sums come from ScalarE's accumulator readout;
# the rest are tensor_reduce'd on the (otherwise idle) DVE.  The last
# groups use RA so the DVE's reduce backlog drains before the finals.
RA_GROUPS = frozenset({10, 11, 12})

# frame row f holds k0 local row f-3 (local rows -1..16 at f=2..19)
EV0 = 2 * W                # eviction start: flat offset of frame row 2
# matmul window for m-tile mt: local rows 4mt..4mt+3 -> frame rows
# 4mt+3..4mt+6 -> flat [32*(4mt+3), +128)
def WOFF(mt):
    return W * (4 * mt + 3)

# tap t = (dh, dw) = (t//3, t%3); copy shift = (dh-1)*32 + (dw-1)
# DoubleRow ktile A: (i, phalf) -> tap [[0, 1], [2, 3]]; B: [[4, 5], [6, 7]]
A_TAPS = [[0, 1], [2, 3]]
B_TAPS = [[4, 5], [6, 7]]


def build():
    nc = bacc.Bacc("TRN2", target_bir_lowering=False, debug=False,
                   num_devices=NCORES)

    xf = nc.dram_tensor("xf", [128, 2, NLOC], BF16, kind="ExternalInput")
    xt = nc.dram_tensor("xt", [128, MT, C], F32, kind="ExternalInput")
    wc = nc.dram_tensor("wc", [128, 2, CH], BF16, kind="ExternalInput")
    bc = nc.dram_tensor("bc", [CH, 1], F32, kind="ExternalInput")
    wk = nc.dram_tensor("wk", [NT, 128, 2, 2, 512], FP8, kind="ExternalInput")
    wk8 = nc.dram_tensor("wk8", [NG, 128, 2, 2, 512], FP8, kind="ExternalInput")
    out = nc.dram_tensor("out", [128, MT, C], F32, kind="ExternalOutput")
    sdbg = nc.dram_tensor("sdbg", [128, MT], F32, kind="ExternalOutput")

    with tile.TileContext(nc) as tc:
        with (
            tc.tile_pool(name="const", bufs=1) as const,
            tc.tile_pool(name="wpool", bufs=12) as wpool,
            tc.tile_pool(name="tpool", bufs=3) as tpool,
            tc.tile_pool(name="ppool", bufs=2, space="PSUM") as ppool,
            tc.tile_pool(name="epool", bufs=4) as epool,
        ):
            def load_wt(n):
                wt = wpool.tile([128, 2, 2, 512], FP8, tag="wt", name=f"wt_{n}")
                nc.sync.dma_start(wt[:], wk.ap()[n])
                return wt

            def load_tail(g):
                tt = tpool.tile([128, 2, 2, 512], FP8, tag="tt", name=f"tt_{g}")
                nc.sync.dma_start(tt[:], wk8.ap()[g])
                return tt

            # ---- input staging: xf kt-halves on the two HW DGE queues
            # (sync + scalar; the pool SWDGE adds ~2us of latency), wc
            # first on scalar so the compress LS is never the gate ----
            x_sb = const.tile([128, 2, NLOC], BF16)
            wc_sb = const.tile([128, 2, CH], BF16)
            bc_sb = const.tile([CH, 1], F32)
            nc.scalar.dma_start(wc_sb[:], wc.ap())
            nc.sync.dma_start(x_sb[:, 0], xf.ap()[:, 0])
            nc.gpsimd.dma_start(x_sb[:, 1], xf.ap()[:, 1])
            nc.scalar.dma_start(bc_sb[:], bc.ap())

            # preload the activation table while input DMAs are in flight
            warm = const.tile([1, 16], F32)
            nc.vector.memset(warm[:], 0.0)
            nc.scalar.activation(warm[:], warm[:], AF.Exp)

            # image strips: U1 = ktile A (taps 0-3), U2 = ktile B (taps 4-7),
            # V8DR = tap 8 in row-tiled DR layout: partition blocks [0:32)
            # and [64:96) both hold (slot i = image channel q+32i).
            U1 = const.tile([128, 2, FRAME], FP8)
            U2 = const.tile([128, 2, FRAME], FP8)
            V8DR = const.tile([128, 2, FRAME], FP8)
            # base strip = U2[0:64, 0] (tap 4, shift 0): zero its frame edges
            nc.vector.memset(U2[0:64, 0, 0:EV0], 0.0)
            nc.vector.memset(U2[0:64, 0, EV0 + NLOC:FRAME], 0.0)

            # ---- compress conv: k0 = w_comp @ x + b_comp, evict as fp8;
            # second half evicted by DVE so the two halves run in parallel ----
            base = U2[0:64, 0, :]
            cps = []
            for nh in range(2):
                ps = ppool.tile([128, 2048], F32, tag="ps", name=f"cps_{nh}")
                for kt in range(2):
                    nc.tensor.matmul(
                        ps[0:CH, 0:NLOC // 2],
                        lhsT=wc_sb[:, kt, :],
                        rhs=x_sb[:, kt, nh * (NLOC // 2):(nh + 1) * (NLOC // 2)],
                        start=(kt == 0), stop=(kt == 1),
                    )
                cps.append(ps)
            nc.scalar.activation(base[:, EV0:EV0 + NLOC // 2],
                                 cps[0][0:CH, 0:NLOC // 2],
                                 AF.Identity, bias=bc_sb[:])
            nc.vector.tensor_scalar_add(base[:, EV0 + NLOC // 2:EV0 + NLOC],
                                        cps[1][0:CH, 0:NLOC // 2], bc_sb[:])

            # ---- shifted copies of the base strip, ALL as engine-compute
            # copies on DVE (tensor_copy) + ScalarE (activation Copy).
            # DGE-issued DMA copies pay ~600ns descriptor generation each
            # AND the Act/Pool DGE queues showed multi-us issue-to-
            # completion-semaphore latencies at startup, which kept the
            # stream waiting on strips; the two compute paths run the 11
            # copies in ~3.5us, in parallel, with fast completion sems.
            # U1 (ktile A) first everywhere: it gates the first matmuls.
            CPY0, CPY1 = 2 * W, 21 * W          # dst copy extent [64, 672)
            def tapshift(t):
                return (t // 3 - 1) * W + (t % 3 - 1)
            def tapcopy_dve(dst, t):
                s = tapshift(t)
                nc.vector.tensor_copy(dst[:, CPY0:CPY1],
                                      base[:, CPY0 + s:CPY1 + s])
            def tapcopy_act(dst, t):
                s = tapshift(t)
                nc.scalar.activation(dst[:, CPY0:CPY1],
                                     base[:, CPY0 + s:CPY1 + s], AF.Copy)
            def colfix(strip, col):
                ap = strip.rearrange("p (r c) -> p r c", c=W)
                nc.vector.memset(ap[:, 3:19, col:col + 1], 0.0)
            S8 = W + 1

            # ScalarE: U1 half 1, then U2 tap 5, then V8DR blocks c/d
            tapcopy_act(U1[64:128, 0, :], 1)
            tapcopy_act(U2[64:128, 0, :], 5)
            for rb, i in ((64, 0), (64, 1)):
                nc.scalar.activation(
                    V8DR[rb:rb + 32, i, CPY0:CPY1],
                    U2[32 * i:32 * i + 32, 0, CPY0 + S8:CPY1 + S8], AF.Copy)
            # DVE: U1 halves 0/2/3 + the U1 fixes, U2 6/7, V8DR a/b, fixes
            tapcopy_dve(U1[0:64, 0, :], 0)
            tapcopy_dve(U1[0:64, 1, :], 2)
            tapcopy_dve(U1[64:128, 1, :], 3)
            colfix(U1[0:64, 0, :], 0)        # tap 0
            colfix(U1[0:64, 1, :], 31)       # tap 2
            colfix(U1[64:128, 1, :], 0)      # tap 3
            tapcopy_dve(U2[0:64, 1, :], 6)
            tapcopy_dve(U2[64:128, 1, :], 7)
            colfix(U2[0:64, 1, :], 0)        # tap 6
            colfix(U2[64:128, 0, :], 31)     # tap 5 (waits ScalarE copy)
            for rb, i in ((0, 0), (0, 1)):
                nc.vector.tensor_copy(
                    V8DR[rb:rb + 32, i, CPY0:CPY1],
                    U2[32 * i:32 * i + 32, 0, CPY0 + S8:CPY1 + S8])
            for i in range(2):               # tap 8 (both DR slots)
                colfix(V8DR[:, i, :], 31)

            # weight prefetch for the first two groups, after the copies.
            # tt0 right after group 0's tiles: the first tail block fires
            # ~2us into the stream and must not sit behind 2MB of weights.
            wts = {n: load_wt(n) for n in range(4)}
            tts = {0: load_tail(0)}
            wts.update({n: load_wt(n) for n in range(4, 8)})
            tts[1] = load_tail(1)

            # ---- PE clock warm-up: the PE ramps from its idle clock over
            # ~4-5us of activity, which the stream start otherwise pays as
            # ~2x matmul slices.  Fill the (otherwise idle) copy-phase PE
            # with dummy DR matmuls on already-resident weight tiles; the
            # results land in unused psum columns and are never read. ----
            for dk in range(18):
                nc.tensor.matmul(cps[dk % 2][:, 1024:1536],
                                 lhsT=wts[0][:, 0, :, 0:128],
                                 rhs=wts[1][:, 0],
                                 start=True, stop=True, perf_mode=DR,
                                 skip_group_check=True)

            # x^T for the output stage: not needed until the end
            xt_sb = const.tile([128, MT, C], F32)
            nc.gpsimd.dma_start(xt_sb[:], xt.ap())

            # ---- big conv + exp + per-pixel partial sums ----
            spart = [const.tile([128, NG], F32, name=f"spart_{mt}")
                     for mt in range(MT)]
            for gi in range(NG):
                T = GT[gi]
                t0 = 4 * gi
                gwt = []
                for t in range(T):
                    n = t0 + t
                    gwt.append(wts.pop(n) if n in wts else load_wt(n))
                tt = tts.pop(gi) if gi in tts else load_tail(gi)
                # prefetch group gi+2
                if gi + 2 < NG:
                    for t in range(GT[gi + 2]):
                        n = 4 * (gi + 2) + t
                        if n not in wts:
                            wts[n] = load_wt(n)
                    if gi + 2 not in tts:
                        tts[gi + 2] = load_tail(gi + 2)
                # m-tiles are processed in pairs: the two m-tiles' tap-8
                # tail blocks are merged into ONE run of 32-row-geometry
                # matmuls between them, so the PE pays the ~140ns
                # array-geometry-switch drain twice per m-tile PAIR (into
                # the tail block, back out to the next 128-row A block)
                # instead of twice per m-tile.
                def ab_block(pt, mt, kt, start, stop):
                    lhs = (U1 if kt == 0 else U2)[:, :, WOFF(mt):WOFF(mt) + 128]
                    for t in range(T):
                        nc.tensor.matmul(pt[:, 512 * t:512 * (t + 1)],
                                         lhsT=lhs, rhs=gwt[t][:, kt],
                                         start=start, stop=stop, perf_mode=DR)

                def tail_block(pt, mt, start, stop):
                    # N=512 DR matmuls (N=1024 fails the ISA's
                    # s3d3_mm_num_elements check); slot s covers tiles
                    # (2s, 2s+1) concurrently on row blocks 0 / 64
                    w0, w1 = WOFF(mt), WOFF(mt) + 128
                    for s in range(2):
                        for rb, t in ((0, 2 * s), (64, 2 * s + 1)):
                            if t >= T:
                                continue
                            nc.tensor.matmul(pt[:, 512 * t:512 * (t + 1)],
                                             lhsT=V8DR[rb:rb + 32, :, w0:w1],
                                             rhs=tt[rb:rb + 32, s, :, :],
                                             start=start, stop=stop,
                                             perf_mode=DR)

                def evict(pt, mt):
                    width = 512 * T
                    et = epool.tile([128, 2048], BF16, tag="et")
                    if gi in RA_GROUPS:
                        nc.scalar.activation(et[:, 0:width], pt[:, 0:width],
                                             AF.Exp, scale=1.0 / WSCALE,
                                             accum_out=spart[mt][:, gi:gi + 1])
                    else:
                        nc.scalar.activation(et[:, 0:width], pt[:, 0:width],
                                             AF.Exp, scale=1.0 / WSCALE)
                        nc.vector.tensor_reduce(
                            spart[mt][:, gi:gi + 1], et[:, 0:width],
                            axis=mybir.AxisListType.X, op=mybir.AluOpType.add,
                        )

                # the last group runs its m-tile pairs in reverse so the
                # finals for m-tiles 2/3 can overlap its 0/1 processing
                for mp in ((1, 0) if gi == NG - 1 else (0, 1)):
                    m0, m1 = 2 * mp, 2 * mp + 1
                    pt0 = ppool.tile([128, 2048], F32, tag="ps",
                                     name=f"pt_{gi}_{m0}")
                    pt1 = ppool.tile([128, 2048], F32, tag="ps",
                                     name=f"pt_{gi}_{m1}")
                    ab_block(pt0, m0, 0, start=True, stop=False)
                    ab_block(pt0, m0, 1, start=False, stop=False)
                    tail_block(pt0, m0, start=False, stop=True)
                    tail_block(pt1, m1, start=True, stop=False)
                    evict(pt0, m0)
                    ab_block(pt1, m1, 0, start=False, stop=False)
                    ab_block(pt1, m1, 1, start=False, stop=True)
                    evict(pt1, m1)

            # ---- per-mt finals: S -> ksum = S/S -> out = x^T * ksum ----
            # pipelined per m-tile so the store overlaps the last groups
            S = const.tile([128, MT], F32)
            rec = const.tile([128, MT], F32)
            ks = const.tile([128, MT], F32)
            ot = const.tile([128, MT, C], F32)
            st_eng = (nc.sync, nc.scalar, nc.gpsimd, nc.sync)
            for mt in (2, 3, 0, 1):
                nc.vector.tensor_reduce(
                    S[:, mt:mt + 1], spart[mt][:, 0:NG],
                    axis=mybir.AxisListType.X, op=mybir.AluOpType.add,
                )
                nc.vector.reciprocal(rec[:, mt:mt + 1], S[:, mt:mt + 1])
                nc.vector.tensor_mul(ks[:, mt:mt + 1], S[:, mt:mt + 1],
                                     rec[:, mt:mt + 1])
                nc.vector.tensor_scalar_mul(
                    ot[:, mt, :], xt_sb[:, mt, :], ks[:, mt:mt + 1],
                )
                st_eng[mt].dma_start(out.ap()[:, mt], ot[:, mt, :])
            nc.scalar.dma_start(sdbg.ap(), S[:])

    nc.compile()
    return nc


_NC = {}


def _get_nc(with_bias=False):
    if with_bias not in _NC:
        _NC[with_bias] = (build_bias if with_bias else build)()
    return _NC[with_bias]


def _pmajor(a, p=128):
    """(k*p, n...) row-major -> (p, k, n...) partition-major."""
    k = a.shape[0] // p
    return np.ascontiguousarray(a.reshape(k, p, *a.shape[1:]).transpose(
        1, 0, *range(2, a.ndim + 1)))


def prep_inputs(x, w_comp, b_comp, w_ker, b_ker):
    import ml_dtypes
    E4 = ml_dtypes.float8_e4m3
    x = np.asarray(x, dtype=np.float32).reshape(C, H, W)
    w_comp = np.asarray(w_comp, dtype=np.float32)
    b_comp = np.asarray(b_comp, dtype=np.float32)
    w_ker = np.asarray(w_ker, dtype=np.float32)
    b_ker = np.asarray(b_ker, dtype=np.float32)
    with_bias = bool(np.any(b_ker)) or bool(np.any(b_comp))
    if with_bias:
        return prep_inputs_bias(x, w_comp, b_comp, w_ker, b_ker), True

    xp = np.zeros((C, H + 2, W), np.float32)
    xp[:, 1:H + 1] = x
    wcT = _pmajor(np.ascontiguousarray(
        w_comp.reshape(CH, C).T).astype(ml_dtypes.bfloat16))
    bcr = np.ascontiguousarray(b_comp.reshape(CH, 1), dtype=np.float32)

    # weights: x16 scale, fp8e4, grouped [nt, p=hi*64+ci, kt, i, n]
    w9 = (w_ker.reshape(OC_TOTAL, CH, 9) * WSCALE).astype(E4)

    in_maps = []
    for core in range(NCORES):
        g, h = core // 2, core % 2
        xfc = _pmajor(np.ascontiguousarray(
            xp[:, 16 * h:16 * h + HLOC].reshape(C, NLOC)
        ).astype(ml_dtypes.bfloat16))
        xtc = _pmajor(np.ascontiguousarray(
            x.reshape(C, H * W)[:, PIX * h:PIX * (h + 1)].T))
        a = w9[GCH * g:GCH * (g + 1)].reshape(NT, 512, CH, 9)
        wkc = np.empty((NT, 128, 2, 2, 512), E4)
        for kt, taps in enumerate((A_TAPS, B_TAPS)):
            for i in range(2):
                for hi in range(2):
                    wkc[:, 64 * hi:64 * (hi + 1), kt, i, :] = (
                        a[:, :, :, taps[i][hi]].transpose(0, 2, 1))
        # tap-8 tails, row-tiled DR layout: slot s covers tiles (2s, 2s+1)
        # at row blocks 0 / 64; wk8[gi, rb+q, s, i, n] = w(tile, n, ch q+32i)
        t8 = a[:, :, :, 8].transpose(0, 2, 1)          # (NT, 64, 512)
        wk8c = np.zeros((NG, 128, 2, 2, 512), E4)
        for gi in range(NG):
            for t in range(GT[gi]):
                s, rb = t // 2, 64 * (t % 2)
                blk = t8[4 * gi + t]                   # (64, 512)
                wk8c[gi, rb:rb + 32, s, 0, :] = blk[0:32]
                wk8c[gi, rb:rb + 32, s, 1, :] = blk[32:64]
        im = {
            "xf": xfc,
            "xt": xtc,
            "wc": wcT,
            "bc": bcr,
            "wk": np.ascontiguousarray(wkc),
            "wk8": wk8c,
        }
        in_maps.append(im)
    return in_maps, False


def assemble(results):
    full = np.empty((C, 4, H, W), dtype=np.float32)
    for core in range(NCORES):
        g, h = core // 2, core % 2
        blk = results[core]["out"]                     # (128, 4, 256)
        pix = blk.transpose(1, 0, 2).reshape(PIX, C)   # (512, 256)
        full[:, g, 16 * h:16 * (h + 1), :] = pix.T.reshape(C, 16, W)
    return full.reshape(1, C, SC * H, SC * W)


def run(in_maps, with_bias=False, trace=False, **kw):
    nc = _get_nc(with_bias)
    return run_bass_kernel_spmd(nc, in_maps, list(range(NCORES)), trace=trace, **kw)


def kernel(x, w_comp, b_comp, w_ker, b_ker):
    in_maps, with_bias = prep_inputs(x, w_comp, b_comp, w_ker, b_ker)
    res = run(in_maps, with_bias)
    return assemble(res.results)


# ---------------------------------------------------------------------------
# Exact nonzero-bias fallback: the v1 kernel (never hit by the benchmark's
# setup_inputs, which has zero biases).  Kept verbatim from the baseline.
# ---------------------------------------------------------------------------

NPAIR = NT // 2


def build_bias():
    with_bias = True
    nc = bacc.Bacc("TRN2", target_bir_lowering=False, debug=False,
                   num_devices=NCORES)

    xf = nc.dram_tensor("xf", [128, 2, NLOC], BF16, kind="ExternalInput")
    xt = nc.dram_tensor("xt", [128, MT, C], F32, kind="ExternalInput")
    wc = nc.dram_tensor("wc", [128, 2, CH], BF16, kind="ExternalInput")
    bc = nc.dram_tensor("bc", [CH, 1], F32, kind="ExternalInput")
    wk = nc.dram_tensor("wk", [NT, 128, 2, 2, 512], FP8, kind="ExternalInput")
    wkt = nc.dram_tensor("wkt", [NT, 65, 512], FP8, kind="ExternalInput")
    hm = nc.dram_tensor("hm", [CH, NLOC], FP8, kind="ExternalInput")
    out = nc.dram_tensor("out", [128, MT, C], F32, kind="ExternalOutput")
    sdbg = nc.dram_tensor("sdbg", [128, MT], F32, kind="ExternalOutput")

    with tile.TileContext(nc) as tc:
        with (
            tc.tile_pool(name="const", bufs=1) as const,
            tc.tile_pool(name="wpool", bufs=8) as wpool,
            tc.tile_pool(name="tpool", bufs=4) as tpool,
            tc.tile_pool(name="ppool", bufs=4, space="PSUM") as ppool,
            tc.tile_pool(name="epool", bufs=3) as epool,
        ):
            def load_wt(n):
                wt = wpool.tile([128, 2, 2, 512], FP8, tag="wt", name=f"wt_{n}")
                nc.sync.dma_start(wt[:], wk.ap()[n])
                return wt

            def load_tail(n):
                tt = tpool.tile([65, 512], FP8, tag="tt", name=f"tt_{n}")
                nc.sync.dma_start(tt[:], wkt.ap()[n])
                return tt

            x_sb = const.tile([128, 2, NLOC], BF16)
            nc.sync.dma_start(x_sb[:], xf.ap())
            wc_sb = const.tile([128, 2, CH], BF16)
            nc.sync.dma_start(wc_sb[:], wc.ap())
            bc_sb = const.tile([CH, 1], F32)
            nc.sync.dma_start(bc_sb[:], bc.ap())

            warm = const.tile([1, 16], F32)
            nc.vector.memset(warm[:], 0.0)
            nc.scalar.activation(warm[:], warm[:], AF.Exp)

            U1 = const.tile([128, 2, FRAME], FP8)
            U2 = const.tile([128, 2, FRAME], FP8)
            V8 = const.tile([128, FRAME], FP8)
            nc.vector.memset(U2[0:64, 0, 0:EV0], 0.0)
            nc.vector.memset(U2[0:64, 0, EV0 + NLOC:FRAME], 0.0)

            base = U2[0:64, 0, :]
            cps = []
            for nh in range(2):
                ps = ppool.tile([128, 1024], F32, tag="ps", name=f"cps_{nh}")
                for kt in range(2):
                    nc.tensor.matmul(
                        ps[0:CH, 0:NLOC // 2],
                        lhsT=wc_sb[:, kt, :],
                        rhs=x_sb[:, kt, nh * (NLOC // 2):(nh + 1) * (NLOC // 2)],
                        start=(kt == 0), stop=(kt == 1),
                    )
                cps.append(ps)
            nc.scalar.activation(base[:, EV0:EV0 + NLOC // 2],
                                 cps[0][0:CH, 0:NLOC // 2],
                                 AF.Identity, bias=bc_sb[:])
            nc.vector.tensor_scalar_add(base[:, EV0 + NLOC // 2:EV0 + NLOC],
                                        cps[1][0:CH, 0:NLOC // 2], bc_sb[:])
            # halo rows hold b_comp instead of the conv's zero padding;
            # mask them
            hm_sb = const.tile([CH, NLOC], FP8)
            nc.gpsimd.dma_start(hm_sb[:], hm.ap())
            nc.vector.tensor_mul(base[:, EV0:EV0 + NLOC],
                                 base[:, EV0:EV0 + NLOC], hm_sb[:])

            CPY0, CPY1 = 2 * W, 21 * W
            def tapcopy(dst, t, eng):
                s = (t // 3 - 1) * W + (t % 3 - 1)
                eng.dma_start(dst[:, CPY0:CPY1], base[:, CPY0 + s:CPY1 + s])
            tapcopy(U1[0:64, 0, :], 0, nc.sync)
            tapcopy(U1[64:128, 0, :], 1, nc.gpsimd)
            tapcopy(U1[0:64, 1, :], 2, nc.scalar)
            tapcopy(U1[64:128, 1, :], 3, nc.sync)
            tapcopy(U2[0:64, 1, :], 6, nc.gpsimd)
            tapcopy(U2[64:128, 0, :], 5, nc.scalar)
            tapcopy(U2[64:128, 1, :], 7, nc.sync)
            tapcopy(V8[0:64, :], 8, nc.gpsimd)
            nc.vector.memset(V8[64:65, :], 1.0)    # bias ones row

            wts = {0: load_wt(0), 1: load_wt(1)}
            tts = {0: load_tail(0), 1: load_tail(1)}

            def colfix(strip, col):
                ap = strip.rearrange("p (r c) -> p r c", c=W)
                nc.vector.memset(ap[:, 3:19, col:col + 1], 0.0)
            colfix(U1[0:64, 0, :], 0)
            colfix(U1[0:64, 1, :], 31)
            colfix(U1[64:128, 1, :], 0)
            colfix(U2[64:128, 0, :], 31)
            colfix(U2[0:64, 1, :], 0)
            colfix(V8[0:64, :], 31)

            xt_sb = const.tile([128, MT, C], F32)
            nc.gpsimd.dma_start(xt_sb[:], xt.ap())

            spart = const.tile([128, MT * NPAIR], F32)
            for pair in range(NPAIR):
                n0, n1 = 2 * pair, 2 * pair + 1
                wt0 = wts.pop(n0) if n0 in wts else load_wt(n0)
                wt1 = wts.pop(n1) if n1 in wts else load_wt(n1)
                tt0 = tts.pop(n0) if n0 in tts else load_tail(n0)
                tt1 = tts.pop(n1) if n1 in tts else load_tail(n1)
                for mt in range(MT):
                    w0, w1 = WOFF(mt), WOFF(mt) + 128
                    lhsA = U1[:, :, w0:w1]
                    lhsB = U2[:, :, w0:w1]
                    pt = ppool.tile([128, 1024], F32, tag="ps",
                                    name=f"pt_{pair}_{mt}")
                    nc.tensor.matmul(pt[:, 0:512], lhsT=lhsA, rhs=wt0[:, 0],
                                     start=True, stop=False, perf_mode=DR)
                    nc.tensor.matmul(pt[:, 512:1024], lhsT=lhsA, rhs=wt1[:, 0],
                                     start=True, stop=False, perf_mode=DR)
                    nc.tensor.matmul(pt[:, 0:512], lhsT=lhsB, rhs=wt0[:, 1],
                                     start=False, stop=False, perf_mode=DR)
                    nc.tensor.matmul(pt[:, 512:1024], lhsT=lhsB, rhs=wt1[:, 1],
                                     start=False, stop=False, perf_mode=DR)
                    nc.tensor.matmul(pt[:, 0:512],
                                     lhsT=V8[0:65, w0:w1], rhs=tt0[:],
                                     start=False, stop=True)
                    nc.tensor.matmul(pt[:, 512:1024],
                                     lhsT=V8[0:65, w0:w1], rhs=tt1[:],
                                     start=False, stop=True)
                    et = epool.tile([128, 1024], BF16, tag="et")
                    idx = mt * NPAIR + pair
                    nc.scalar.activation(et[:], pt[:], AF.Exp,
                                         scale=1.0 / WSCALE,
                                         accum_out=spart[:, idx:idx + 1])

            S = const.tile([128, MT], F32)
            rec = const.tile([128, MT], F32)
            ks = const.tile([128, MT], F32)
            ot = const.tile([128, MT, C], F32)
            st_eng = (nc.sync, nc.scalar, nc.gpsimd, nc.sync)
            for mt in range(MT):
                nc.vector.tensor_reduce(
                    S[:, mt:mt + 1], spart[:, mt * NPAIR:(mt + 1) * NPAIR],
                    axis=mybir.AxisListType.X, op=mybir.AluOpType.add,
                )
                nc.vector.reciprocal(rec[:, mt:mt + 1], S[:, mt:mt + 1])
                nc.vector.tensor_mul(ks[:, mt:mt + 1], S[:, mt:mt + 1],
                                     rec[:, mt:mt + 1])
                nc.vector.tensor_scalar_mul(
                    ot[:, mt, :], xt_sb[:, mt, :], ks[:, mt:mt + 1],
                )
                st_eng[mt].dma_start(out.ap()[:, mt], ot[:, mt, :])
            nc.scalar.dma_start(sdbg.ap(), S[:])

    nc.compile()
    return nc


def prep_inputs_bias(x, w_comp, b_comp, w_ker, b_ker):
    import ml_dtypes
    E4 = ml_dtypes.float8_e4m3
    xp = np.zeros((C, H + 2, W), np.float32)
    xp[:, 1:H + 1] = x
    wcT = _pmajor(np.ascontiguousarray(
        w_comp.reshape(CH, C).T).astype(ml_dtypes.bfloat16))
    bcr = np.ascontiguousarray(b_comp.reshape(CH, 1), dtype=np.float32)

    w9 = (w_ker.reshape(OC_TOTAL, CH, 9) * WSCALE).astype(E4)
    bk16 = (b_ker * WSCALE).astype(E4)

    in_maps = []
    for core in range(NCORES):
        g, h = core // 2, core % 2
        xfc = _pmajor(np.ascontiguousarray(
            xp[:, 16 * h:16 * h + HLOC].reshape(C, NLOC)
        ).astype(ml_dtypes.bfloat16))
        xtc = _pmajor(np.ascontiguousarray(
            x.reshape(C, H * W)[:, PIX * h:PIX * (h + 1)].T))
        a = w9[GCH * g:GCH * (g + 1)].reshape(NT, 512, CH, 9)
        wkc = np.empty((NT, 128, 2, 2, 512), E4)
        for kt, taps in enumerate((A_TAPS, B_TAPS)):
            for i in range(2):
                for hi in range(2):
                    wkc[:, 64 * hi:64 * (hi + 1), kt, i, :] = (
                        a[:, :, :, taps[i][hi]].transpose(0, 2, 1))
        t8 = a[:, :, :, 8].transpose(0, 2, 1)          # (NT, 64, 512)
        wktc = np.empty((NT, 65, 512), E4)
        wktc[:, 0:64] = t8
        wktc[:, 64] = bk16[GCH * g:GCH * (g + 1)].reshape(NT, 512)
        hmv = np.ones((CH, HLOC, W), np.float32)
        hmv[:, 0 if h == 0 else HLOC - 1] = 0.0
        im = {
            "xf": xfc,
            "xt": xtc,
            "wc": wcT,
            "bc": bcr,
            "wk": np.ascontiguousarray(wkc),
            "wkt": wktc,
            "hm": hmv.reshape(CH, NLOC).astype(E4),
        }
        in_maps.append(im)
    return in_maps


# revision 36
# speedup vs baseline: 1.0336x; 1.0336x over previous
"""CARAFE kernel for 8 TRN2 NeuronCores (Bass/Tile, SPMD).

Math (see reference):
  k0   = w_comp @ x + b_comp                 (64, 32, 32)      1x1 conv
  kc   = w_ker (*) k0 + b_ker                (102400, 32, 32)  3x3 conv, pad 1
  k    = softmax(kc.reshape(4, 25600, H, W), axis=1)
  ksum = k.sum(axis=1)                       (4, 32, 32)       == S/S (==1+eps)
  out  = (x[:, :, None] * ksum[:, None]).reshape(1, 256, 64, 64)

Sharding: core c = (g, h) with g = c//2 (softmax/scale group), h = c%2
(pixel half: image rows 16h..16h+16).  Each core computes its group's
FULL 25600 conv channels for its OWN 512 pixels, so the softmax group
sum S is core-local -- no collective at all.

v2 layout (vs the 161.5us baseline, which was ScalarE-bound at
1295ns/pair-iteration: 1114ns EXP[128,1024] + 181ns accumulator
readout, with the PE stretched to match):

  * Channel tiles are processed in GROUPS of 4 (2 psum banks per 512
    tile -> [128, 2048] psum spanning 4 banks, 2 groups in flight =
    all 8 banks).  One 2048-wide Exp eviction per group amortizes the
    ScalarE per-instruction overhead (~300cy) 2x better, and only a
    subset of groups pay the 181ns ACTIVATION_READ_ACCUMULATOR --
    the rest get their per-pixel partial sums from a DVE
    tensor_reduce of the (otherwise-dead) exp output tile, keeping
    the otherwise-idle Vector engine busy instead of ScalarE.
  * tap-8 tails run as concurrent row-tiled DR matmuls, 32 array rows
    at base partitions 0 and 64 (offset 96 is the unusable PE quadrant
    3, so 4-way packing is not possible): two tail slots per 4-tile
    group, each slot covering 2 tiles concurrently with K=64 via
    DoubleRow on 32 partitions.
  * The im2col trick is unchanged: the compressed image (18 rows incl
    halo) lives in 23-row x 32-col zero-framed fp8 strips; each tap is
    a flat-shifted SBUF->SBUF DMA copy (shift (dh-1)*32+(dw-1)), with
    row-crossing leak columns zeroed by tiny DVE memsets.  The tail
    strip V8DR holds tap 8 in 4-way row-tiled DR layout: partition
    32t+q, slot i = image channel q+32i (each 32-row block is a plain
    partition-offset copy of half the base strip).
  * Startup: the compress-conv input xf is split across two DGE
    queues (sync + pool) so the 294KB load halves; weight prefetch is
    issued on the sync queue strictly after the strip copies so the
    copy DMAs (which gate the first matmul) are never stuck behind
    512KB of weights (baseline lesson).
  * Finals are per-mt pipelined; spart is 4 per-mt tiles so each
    m-tile's S reduce only waits on its own last eviction; the 4
    output stores go out on 4 different engine DGEs.
  * Weights are scaled x16 on the host before fp8e4 quantization
    (their 0.05 sigma sits in e4m3's subnormal range); the Exp
    eviction's activation scale of 1/16 undoes it exactly.  Softmax
    sums are divided by themselves (ksum == 1 in exact arithmetic), so
    conv precision does not reach the output.  A separate exact
    variant (the v1 kernel, K=65 ones row + halo mask) is built
    lazily if nonzero biases are ever passed.
"""

import numpy as np

import concourse.bass as bass
import concourse.mybir as mybir
import concourse.tile as tile
from concourse import bacc
from concourse.bass_utils import run_bass_kernel_spmd

F32 = mybir.dt.float32
BF16 = mybir.dt.bfloat16
FP8 = mybir.dt.float8e4
AF = mybir.ActivationFunctionType
DR = mybir.MatmulPerfMode.DoubleRow

# Problem constants
C, H, W = 256, 32, 32
CH = 64                    # compressed channels
SC = 2                     # upsample scale
OC_TOTAL = 102400
NCORES = 8
GCH = OC_TOTAL // 4        # 25600 channels per softmax group (= per core)
NT = GCH // 512            # 50 channel tiles of 512
NG = 13                    # channel-tile groups per m-tile: 12x4 + 1x2
GT = [4] * 12 + [2]        # tiles per group
PIX = 512                  # pixels per core (16 image rows)
MT = PIX // 128            # 4 pixel tiles of 128 (4 image rows each)
HLOC = 18                  # local k0 rows incl 1-row halo each side
NLOC = HLOC * W            # 576 compress-conv pixels
FROWS = 23                 # zero-framed strip rows
FRAME = FROWS * W          # 736 bytes per image copy (div by 16)
WSCALE = 16.0              # host weight scale, undone by Exp's 1/16

# groups whose partial sums come from ScalarE's accumulator readout;
# the rest are tensor_reduce'd on the (otherwise idle) DVE.  The last
# groups use RA so the DVE's reduce backlog drains before the finals.
RA_GROUPS = frozenset({10, 11, 12})

# frame row f holds k0 local row f-3 (local rows -1..16 at f=2..19)
EV0 = 2 * W                # eviction start: flat offset of frame row 2
# matmul window for m-tile mt: local rows 4mt..4mt+3 -> frame rows
# 4mt+3..4mt+6 -> flat [32*(4mt+3), +128)
def WOFF(mt):
    return W * (4 * mt + 3)

# tap t = (dh, dw) = (t//3, t%3); copy shift = (dh-1)*32 + (dw-1)
# DoubleRow ktile A: (i, phalf) -> tap [[0, 1], [2, 3]]; B: [[4, 5], [6, 7]]
A_TAPS = [[0, 1], [2, 3]]
B_TAPS = [[4, 5], [6, 7]]


def build():
    nc = bacc.Bacc("TRN2", target_bir_lowering=False, debug=False,
                   num_devices=NCORES)

    xf = nc.dram_tensor("xf", [128, 2, NLOC], BF16, kind="ExternalInput")
    xt = nc.dram_tensor("xt", [128, MT, C], F32, kind="ExternalInput")
    wc = nc.dram_tensor("wc", [128, 2, CH], BF16, kind="ExternalInput")
    bc = nc.dram_tensor("bc", [CH, 1], F32, kind="ExternalInput")
    wk = nc.dram_tensor("wk", [NT, 128, 2, 2, 512], FP8, kind="ExternalInput")
    wk8 = nc.dram_tensor("wk8", [NG, 128, 2, 2, 512], FP8, kind="ExternalInput")
    out = nc.dram_tensor("out", [128, MT, C], F32, kind="ExternalOutput")
    sdbg = nc.dram_tensor("sdbg", [128, MT], F32, kind="ExternalOutput")

    with tile.TileContext(nc) as tc:
        with (
            tc.tile_pool(name="const", bufs=1) as const,
            tc.tile_pool(name="wpool", bufs=12) as wpool,
            tc.tile_pool(name="tpool", bufs=3) as tpool,
            tc.tile_pool(name="ppool", bufs=2, space="PSUM") as ppool,
            tc.tile_pool(name="epool", bufs=4) as epool,
        ):
            def load_wt(n):
                wt = wpool.tile([128, 2, 2, 512], FP8, tag="wt", name=f"wt_{n}")
                nc.sync.dma_start(wt[:], wk.ap()[n])
                return wt

            def load_tail(g):
                tt = tpool.tile([128, 2, 2, 512], FP8, tag="tt", name=f"tt_{g}")
                nc.sync.dma_start(tt[:], wk8.ap()[g])
                return tt

            # ---- input staging: xf kt-halves on the two HW DGE queues
            # (sync + scalar; the pool SWDGE adds ~2us of latency), wc
            # first on scalar so the compress LS is never the gate ----
            x_sb = const.tile([128, 2, NLOC], BF16)
            wc_sb = const.tile([128, 2, CH], BF16)
            bc_sb = const.tile([CH, 1], F32)
            nc.scalar.dma_start(wc_sb[:], wc.ap())
            nc.sync.dma_start(x_sb[:, 0], xf.ap()[:, 0])
            nc.gpsimd.dma_start(x_sb[:, 1], xf.ap()[:, 1])
            nc.scalar.dma_start(bc_sb[:], bc.ap())

            # preload the activation table while input DMAs are in flight
            warm = const.tile([1, 16], F32)
            nc.vector.memset(warm[:], 0.0)
            nc.scalar.activation(warm[:], warm[:], AF.Exp)

            # image strips: U1 = ktile A (taps 0-3), U2 = ktile B (taps 4-7),
            # V8DR = tap 8 in row-tiled DR layout: partition blocks [0:32)
            # and [64:96) both hold (slot i = image channel q+32i).
            U1 = const.tile([128, 2, FRAME], FP8)
            U2 = const.tile([128, 2, FRAME], FP8)
            V8DR = const.tile([128, 2, FRAME], FP8)
            # base strip = U2[0:64, 0] (tap 4, shift 0): zero its frame edges
            nc.vector.memset(U2[0:64, 0, 0:EV0], 0.0)
            nc.vector.memset(U2[0:64, 0, EV0 + NLOC:FRAME], 0.0)

            # ---- compress conv: k0 = w_comp @ x + b_comp, evict as fp8;
            # second half evicted by DVE so the two halves run in parallel ----
            base = U2[0:64, 0, :]
            cps = []
            for nh in range(2):
                ps = ppool.tile([128, 2048], F32, tag="ps", name=f"cps_{nh}")
                for kt in range(2):
                    nc.tensor.matmul(
                        ps[0:CH, 0:NLOC // 2],
                        lhsT=wc_sb[:, kt, :],
                        rhs=x_sb[:, kt, nh * (NLOC // 2):(nh + 1) * (NLOC // 2)],
                        start=(kt == 0), stop=(kt == 1),
                    )
                cps.append(ps)
            nc.scalar.activation(base[:, EV0:EV0 + NLOC // 2],
                                 cps[0][0:CH, 0:NLOC // 2],
                                 AF.Identity, bias=bc_sb[:])
            nc.vector.tensor_scalar_add(base[:, EV0 + NLOC // 2:EV0 + NLOC],
                                        cps[1][0:CH, 0:NLOC // 2], bc_sb[:])

            # ---- shifted copies of the base strip, ALL as engine-compute
            # copies on DVE (tensor_copy) + ScalarE (activation Copy).
            # DGE-issued DMA copies pay ~600ns descriptor generation each
            # AND the Act/Pool DGE queues showed multi-us issue-to-
            # completion-semaphore latencies at startup, which kept the
            # stream waiting on strips; the two compute paths run the 11
            # copies in ~3.5us, in parallel, with fast completion sems.
            # U1 (ktile A) first everywhere: it gates the first matmuls.
            CPY0, CPY1 = 2 * W, 21 * W          # dst copy extent [64, 672)
            def tapshift(t):
                return (t // 3 - 1) * W + (t % 3 - 1)
            def tapcopy_dve(dst, t):
                s = tapshift(t)
                nc.vector.tensor_copy(dst[:, CPY0:CPY1],
                                      base[:, CPY0 + s:CPY1 + s])
            def tapcopy_act(dst, t):
                s = tapshift(t)
                nc.scalar.activation(dst[:, CPY0:CPY1],
                                     base[:, CPY0 + s:CPY1 + s], AF.Copy)
            def colfix(strip, col):
                ap = strip.rearrange("p (r c) -> p r c", c=W)
                nc.vector.memset(ap[:, 3:19, col:col + 1], 0.0)
            S8 = W + 1

            # ScalarE: U1 half 1, then U2 tap 5, then V8DR blocks c/d
            tapcopy_act(U1[64:128, 0, :], 1)
            tapcopy_act(U2[64:128, 0, :], 5)
            for rb, i in ((64, 0), (64, 1)):
                nc.scalar.activation(
                    V8DR[rb:rb + 32, i, CPY0:CPY1],
                    U2[32 * i:32 * i + 32, 0, CPY0 + S8:CPY1 + S8], AF.Copy)
            # DVE: U1 halves 0/2/3 + the U1 fixes, U2 6/7, V8DR a/b, fixes
            tapcopy_dve(U1[0:64, 0, :], 0)
            tapcopy_dve(U1[0:64, 1, :], 2)
            tapcopy_dve(U1[64:128, 1, :], 3)
            # U1 leak fixes on the (otherwise idle) gpsimd engine so the
            # DVE copy chain is not extended by them
            def colfix_gp(strip, col):
                ap = strip.rearrange("p (r c) -> p r c", c=W)
                nc.gpsimd.memset(ap[:, 3:19, col:col + 1], 0.0)
            colfix_gp(U1[0:64, 0, :], 0)     # tap 0
            colfix_gp(U1[0:64, 1, :], 31)    # tap 2
            colfix_gp(U1[64:128, 1, :], 0)   # tap 3
            tapcopy_dve(U2[0:64, 1, :], 6)
            tapcopy_dve(U2[64:128, 1, :], 7)
            colfix(U2[0:64, 1, :], 0)        # tap 6
            colfix(U2[64:128, 0, :], 31)     # tap 5 (waits ScalarE copy)
            for rb, i in ((0, 0), (0, 1)):
                nc.vector.tensor_copy(
                    V8DR[rb:rb + 32, i, CPY0:CPY1],
                    U2[32 * i:32 * i + 32, 0, CPY0 + S8:CPY1 + S8])
            for i in range(2):               # tap 8 (both DR slots)
                colfix(V8DR[:, i, :], 31)

            # weight prefetch for the first two groups, after the copies.
            # tt0 right after group 0's tiles: the first tail block fires
            # ~2us into the stream and must not sit behind 2MB of weights.
            wts = {n: load_wt(n) for n in range(4)}
            tts = {0: load_tail(0)}
            wts.update({n: load_wt(n) for n in range(4, 8)})
            tts[1] = load_tail(1)



            # x^T for the output stage: not needed until the end
            xt_sb = const.tile([128, MT, C], F32)
            nc.gpsimd.dma_start(xt_sb[:], xt.ap())

            # ---- big conv + exp + per-pixel partial sums ----
            spart = [const.tile([128, NG], F32, name=f"spart_{mt}")
                     for mt in range(MT)]
            for gi in range(NG):
                T = GT[gi]
                t0 = 4 * gi
                gwt = []
                for t in range(T):
                    n = t0 + t
                    gwt.append(wts.pop(n) if n in wts else load_wt(n))
                tt = tts.pop(gi) if gi in tts else load_tail(gi)
                # prefetch group gi+2
                if gi + 2 < NG:
                    for t in range(GT[gi + 2]):
                        n = 4 * (gi + 2) + t
                        if n not in wts:
                            wts[n] = load_wt(n)
                    if gi + 2 not in tts:
                        tts[gi + 2] = load_tail(gi + 2)
                # m-tiles are processed in pairs: the two m-tiles' tap-8
                # tail blocks are merged into ONE run of 32-row-geometry
                # matmuls between them, so the PE pays the ~140ns
                # array-geometry-switch drain twice per m-tile PAIR (into
                # the tail block, back out to the next 128-row A block)
                # instead of twice per m-tile.
                def ab_block(pt, mt, kt, start, stop):
                    lhs = (U1 if kt == 0 else U2)[:, :, WOFF(mt):WOFF(mt) + 128]
                    for t in range(T):
                        nc.tensor.matmul(pt[:, 512 * t:512 * (t + 1)],
                                         lhsT=lhs, rhs=gwt[t][:, kt],
                                         start=start, stop=stop, perf_mode=DR)

                def tail_block(pt, mt, start, stop):
                    # N=512 DR matmuls (N=1024 fails the ISA's
                    # s3d3_mm_num_elements check); slot s covers tiles
                    # (2s, 2s+1) concurrently on row blocks 0 / 64
                    w0, w1 = WOFF(mt), WOFF(mt) + 128
                    for s in range(2):
                        for rb, t in ((0, 2 * s), (64, 2 * s + 1)):
                            if t >= T:
                                continue
                            nc.tensor.matmul(pt[:, 512 * t:512 * (t + 1)],
                                             lhsT=V8DR[rb:rb + 32, :, w0:w1],
                                             rhs=tt[rb:rb + 32, s, :, :],
                                             start=start, stop=stop,
                                             perf_mode=DR)

                def evict(pt, mt):
                    width = 512 * T
                    et = epool.tile([128, 2048], BF16, tag="et")
                    if gi in RA_GROUPS:
                        nc.scalar.activation(et[:, 0:width], pt[:, 0:width],
                                             AF.Exp, scale=1.0 / WSCALE,
                                             accum_out=spart[mt][:, gi:gi + 1])
                    else:
                        nc.scalar.activation(et[:, 0:width], pt[:, 0:width],
                                             AF.Exp, scale=1.0 / WSCALE)
                        nc.vector.tensor_reduce(
                            spart[mt][:, gi:gi + 1], et[:, 0:width],
                            axis=mybir.AxisListType.X, op=mybir.AluOpType.add,
                        )

                # the last group runs its m-tile pairs in reverse so the
                # finals for m-tiles 2/3 can overlap its 0/1 processing
                for mp in ((1, 0) if gi == NG - 1 else (0, 1)):
                    m0, m1 = 2 * mp, 2 * mp + 1
                    pt0 = ppool.tile([128, 2048], F32, tag="ps",
                                     name=f"pt_{gi}_{m0}")
                    pt1 = ppool.tile([128, 2048], F32, tag="ps",
                                     name=f"pt_{gi}_{m1}")
                    ab_block(pt0, m0, 0, start=True, stop=False)
                    ab_block(pt0, m0, 1, start=False, stop=False)
                    tail_block(pt0, m0, start=False, stop=True)
                    tail_block(pt1, m1, start=True, stop=False)
                    evict(pt0, m0)
                    ab_block(pt1, m1, 0, start=False, stop=False)
                    ab_block(pt1, m1, 1, start=False, stop=True)
                    evict(pt1, m1)

            # ---- per-mt finals: S -> ksum = S/S -> out = x^T * ksum ----
            # pipelined per m-tile so the store overlaps the last groups
            S = const.tile([128, MT], F32)
            rec = const.tile([128, MT], F32)
            ks = const.tile([128, MT], F32)
            ot = const.tile([128, MT, C], F32)
            # end-of-kernel stores avoid the scalar (Act) DGE queue, whose
            # issue-to-completion latency measured several us -- the NEFF
            # end barrier would wait on it
            st_eng = (nc.sync, nc.sync, nc.gpsimd, nc.sync)
            for mt in (2, 3, 0, 1):
                nc.vector.tensor_reduce(
                    S[:, mt:mt + 1], spart[mt][:, 0:NG],
                    axis=mybir.AxisListType.X, op=mybir.AluOpType.add,
                )
                nc.vector.reciprocal(rec[:, mt:mt + 1], S[:, mt:mt + 1])
                nc.vector.tensor_mul(ks[:, mt:mt + 1], S[:, mt:mt + 1],
                                     rec[:, mt:mt + 1])
                nc.vector.tensor_scalar_mul(
                    ot[:, mt, :], xt_sb[:, mt, :], ks[:, mt:mt + 1],
                )
                st_eng[mt].dma_start(out.ap()[:, mt], ot[:, mt, :])
            nc.sync.dma_start(sdbg.ap(), S[:])

    nc.compile()
    return nc


_NC = {}


def _get_nc(with_bias=False):
    if with_bias not in _NC:
        _NC[with_bias] = (build_bias if with_bias else build)()
    return _NC[with_bias]


def _pmajor(a, p=128):
    """(k*p, n...) row-major -> (p, k, n...) partition-major."""
    k = a.shape[0] // p
    return np.ascontiguousarray(a.reshape(k, p, *a.shape[1:]).transpose(
        1, 0, *range(2, a.ndim + 1)))


def prep_inputs(x, w_comp, b_comp, w_ker, b_ker):
    import ml_dtypes
    E4 = ml_dtypes.float8_e4m3
    x = np.asarray(x, dtype=np.float32).reshape(C, H, W)
    w_comp = np.asarray(w_comp, dtype=np.float32)
    b_comp = np.asarray(b_comp, dtype=np.float32)
    w_ker = np.asarray(w_ker, dtype=np.float32)
    b_ker = np.asarray(b_ker, dtype=np.float32)
    with_bias = bool(np.any(b_ker)) or bool(np.any(b_comp))
    if with_bias:
        return prep_inputs_bias(x, w_comp, b_comp, w_ker, b_ker), True

    xp = np.zeros((C, H + 2, W), np.float32)
    xp[:, 1:H + 1] = x
    wcT = _pmajor(np.ascontiguousarray(
        w_comp.reshape(CH, C).T).astype(ml_dtypes.bfloat16))
    bcr = np.ascontiguousarray(b_comp.reshape(CH, 1), dtype=np.float32)

    # weights: x16 scale, fp8e4, grouped [nt, p=hi*64+ci, kt, i, n]
    w9 = (w_ker.reshape(OC_TOTAL, CH, 9) * WSCALE).astype(E4)

    in_maps = []
    for core in range(NCORES):
        g, h = core // 2, core % 2
        xfc = _pmajor(np.ascontiguousarray(
            xp[:, 16 * h:16 * h + HLOC].reshape(C, NLOC)
        ).astype(ml_dtypes.bfloat16))
        xtc = _pmajor(np.ascontiguousarray(
            x.reshape(C, H * W)[:, PIX * h:PIX * (h + 1)].T))
        a = w9[GCH * g:GCH * (g + 1)].reshape(NT, 512, CH, 9)
        wkc = np.empty((NT, 128, 2, 2, 512), E4)
        for kt, taps in enumerate((A_TAPS, B_TAPS)):
            for i in range(2):
                for hi in range(2):
                    wkc[:, 64 * hi:64 * (hi + 1), kt, i, :] = (
                        a[:, :, :, taps[i][hi]].transpose(0, 2, 1))
        # tap-8 tails, row-tiled DR layout: slot s covers tiles (2s, 2s+1)
        # at row blocks 0 / 64; wk8[gi, rb+q, s, i, n] = w(tile, n, ch q+32i)
        t8 = a[:, :, :, 8].transpose(0, 2, 1)          # (NT, 64, 512)
        wk8c = np.zeros((NG, 128, 2, 2, 512), E4)
        for gi in range(NG):
            for t in range(GT[gi]):
                s, rb = t // 2, 64 * (t % 2)
                blk = t8[4 * gi + t]                   # (64, 512)
                wk8c[gi, rb:rb + 32, s, 0, :] = blk[0:32]
                wk8c[gi, rb:rb + 32, s, 1, :] = blk[32:64]
        im = {
            "xf": xfc,
            "xt": xtc,
            "wc": wcT,
            "bc": bcr,
            "wk": np.ascontiguousarray(wkc),
            "wk8": wk8c,
        }
        in_maps.append(im)
    return in_maps, False


def assemble(results):
    full = np.empty((C, 4, H, W), dtype=np.float32)
    for core in range(NCORES):
        g, h = core // 2, core % 2
        blk = results[core]["out"]                     # (128, 4, 256)
        pix = blk.transpose(1, 0, 2).reshape(PIX, C)   # (512, 256)
        full[:, g, 16 * h:16 * (h + 1), :] = pix.T.reshape(C, 16, W)
    return full.reshape(1, C, SC * H, SC * W)


def run(in_maps, with_bias=False, trace=False, **kw):
    nc = _get_nc(with_bias)
    return run_bass_kernel_spmd(nc, in_maps, list(range(NCORES)), trace=trace, **kw)


def kernel(x, w_comp, b_comp, w_ker, b_ker):
    in_maps, with_bias = prep_inputs(x, w_comp, b_comp, w_ker, b_ker)
    res = run(in_maps, with_bias)
    return assemble(res.results)


# ---------------------------------------------------------------------------
# Exact nonzero-bias fallback: the v1 kernel (never hit by the benchmark's
# setup_inputs, which has zero biases).  Kept verbatim from the baseline.
# ---------------------------------------------------------------------------

NPAIR = NT // 2


def build_bias():
    with_bias = True
    nc = bacc.Bacc("TRN2", target_bir_lowering=False, debug=False,
                   num_devices=NCORES)

    xf = nc.dram_tensor("xf", [128, 2, NLOC], BF16, kind="ExternalInput")
    xt = nc.dram_tensor("xt", [128, MT, C], F32, kind="ExternalInput")
    wc = nc.dram_tensor("wc", [128, 2, CH], BF16, kind="ExternalInput")
    bc = nc.dram_tensor("bc", [CH, 1], F32, kind="ExternalInput")
    wk = nc.dram_tensor("wk", [NT, 128, 2, 2, 512], FP8, kind="ExternalInput")
    wkt = nc.dram_tensor("wkt", [NT, 65, 512], FP8, kind="ExternalInput")
    hm = nc.dram_tensor("hm", [CH, NLOC], FP8, kind="ExternalInput")
    out = nc.dram_tensor("out", [128, MT, C], F32, kind="ExternalOutput")
    sdbg = nc.dram_tensor("sdbg", [128, MT], F32, kind="ExternalOutput")

    with tile.TileContext(nc) as tc:
        with (
            tc.tile_pool(name="const", bufs=1) as const,
            tc.tile_pool(name="wpool", bufs=8) as wpool,
            tc.tile_pool(name="tpool", bufs=4) as tpool,
            tc.tile_pool(name="ppool", bufs=4, space="PSUM") as ppool,
            tc.tile_pool(name="epool", bufs=3) as epool,
        ):
            def load_wt(n):
                wt = wpool.tile([128, 2, 2, 512], FP8, tag="wt", name=f"wt_{n}")
                nc.sync.dma_start(wt[:], wk.ap()[n])
                return wt

            def load_tail(n):
                tt = tpool.tile([65, 512], FP8, tag="tt", name=f"tt_{n}")
                nc.sync.dma_start(tt[:], wkt.ap()[n])
                return tt

            x_sb = const.tile([128, 2, NLOC], BF16)
            nc.sync.dma_start(x_sb[:], xf.ap())
            wc_sb = const.tile([128, 2, CH], BF16)
            nc.sync.dma_start(wc_sb[:], wc.ap())
            bc_sb = const.tile([CH, 1], F32)
            nc.sync.dma_start(bc_sb[:], bc.ap())

            warm = const.tile([1, 16], F32)
            nc.vector.memset(warm[:], 0.0)
            nc.scalar.activation(warm[:], warm[:], AF.Exp)

            U1 = const.tile([128, 2, FRAME], FP8)
            U2 = const.tile([128, 2, FRAME], FP8)
            V8 = const.tile([128, FRAME], FP8)
            nc.vector.memset(U2[0:64, 0, 0:EV0], 0.0)
            nc.vector.memset(U2[0:64, 0, EV0 + NLOC:FRAME], 0.0)

            base = U2[0:64, 0, :]
            cps = []
            for nh in range(2):
                ps = ppool.tile([128, 1024], F32, tag="ps", name=f"cps_{nh}")
                for kt in range(2):
                    nc.tensor.matmul(
                        ps[0:CH, 0:NLOC // 2],
                        lhsT=wc_sb[:, kt, :],
                        rhs=x_sb[:, kt, nh * (NLOC // 2):(nh + 1) * (NLOC // 2)],
                        start=(kt == 0), stop=(kt == 1),
                    )
                cps.append(ps)
            nc.scalar.activation(base[:, EV0:EV0 + NLOC // 2],
                                 cps[0][0:CH, 0:NLOC // 2],
                                 AF.Identity, bias=bc_sb[:])
            nc.vector.tensor_scalar_add(base[:, EV0 + NLOC // 2:EV0 + NLOC],
                                        cps[1][0:CH, 0:NLOC // 2], bc_sb[:])
            # halo rows hold b_comp instead of the conv's zero padding;
            # mask them
            hm_sb = const.tile([CH, NLOC], FP8)
            nc.gpsimd.dma_start(hm_sb[:], hm.ap())
            nc.vector.tensor_mul(base[:, EV0:EV0 + NLOC],
                                 base[:, EV0:EV0 + NLOC], hm_sb[:])

            CPY0, CPY1 = 2 * W, 21 * W
            def tapcopy(dst, t, eng):
                s = (t // 3 - 1) * W + (t % 3 - 1)
                eng.dma_start(dst[:, CPY0:CPY1], base[:, CPY0 + s:CPY1 + s])
            tapcopy(U1[0:64, 0, :], 0, nc.sync)
            tapcopy(U1[64:128, 0, :], 1, nc.gpsimd)
            tapcopy(U1[0:64, 1, :], 2, nc.scalar)
            tapcopy(U1[64:128, 1, :], 3, nc.sync)
            tapcopy(U2[0:64, 1, :], 6, nc.gpsimd)
            tapcopy(U2[64:128, 0, :], 5, nc.scalar)
            tapcopy(U2[64:128, 1, :], 7, nc.sync)
            tapcopy(V8[0:64, :], 8, nc.gpsimd)
            nc.vector.memset(V8[64:65, :], 1.0)    # bias ones row

            wts = {0: load_wt(0), 1: load_wt(1)}
            tts = {0: load_tail(0), 1: load_tail(1)}

            def colfix(strip, col):
                ap = strip.rearrange("p (r c) -> p r c", c=W)
                nc.vector.memset(ap[:, 3:19, col:col + 1], 0.0)
            colfix(U1[0:64, 0, :], 0)
            colfix(U1[0:64, 1, :], 31)
            colfix(U1[64:128, 1, :], 0)
            colfix(U2[64:128, 0, :], 31)
            colfix(U2[0:64, 1, :], 0)
            colfix(V8[0:64, :], 31)

            xt_sb = const.tile([128, MT, C], F32)
            nc.gpsimd.dma_start(xt_sb[:], xt.ap())

            spart = const.tile([128, MT * NPAIR], F32)
            for pair in range(NPAIR):
                n0, n1 = 2 * pair, 2 * pair + 1
                wt0 = wts.pop(n0) if n0 in wts else load_wt(n0)
                wt1 = wts.pop(n1) if n1 in wts else load_wt(n1)
                tt0 = tts.pop(n0) if n0 in tts else load_tail(n0)
                tt1 = tts.pop(n1) if n1 in tts else load_tail(n1)
                for mt in range(MT):
                    w0, w1 = WOFF(mt), WOFF(mt) + 128
                    lhsA = U1[:, :, w0:w1]
                    lhsB = U2[:, :, w0:w1]
                    pt = ppool.tile([128, 1024], F32, tag="ps",
                                    name=f"pt_{pair}_{mt}")
                    nc.tensor.matmul(pt[:, 0:512], lhsT=lhsA, rhs=wt0[:, 0],
                                     start=True, stop=False, perf_mode=DR)
                    nc.tensor.matmul(pt[:, 512:1024], lhsT=lhsA, rhs=wt1[:, 0],
                                     start=True, stop=False, perf_mode=DR)
                    nc.tensor.matmul(pt[:, 0:512], lhsT=lhsB, rhs=wt0[:, 1],
                                     start=False, stop=False, perf_mode=DR)
                    nc.tensor.matmul(pt[:, 512:1024], lhsT=lhsB, rhs=wt1[:, 1],
                                     start=False, stop=False, perf_mode=DR)
                    nc.tensor.matmul(pt[:, 0:512],
                                     lhsT=V8[0:65, w0:w1], rhs=tt0[:],
                                     start=False, stop=True)
                    nc.tensor.matmul(pt[:, 512:1024],
                                     lhsT=V8[0:65, w0:w1], rhs=tt1[:],
                                     start=False, stop=True)
                    et = epool.tile([128, 1024], BF16, tag="et")
                    idx = mt * NPAIR + pair
                    nc.scalar.activation(et[:], pt[:], AF.Exp,
                                         scale=1.0 / WSCALE,
                                         accum_out=spart[:, idx:idx + 1])

            S = const.tile([128, MT], F32)
            rec = const.tile([128, MT], F32)
            ks = const.tile([128, MT], F32)
            ot = const.tile([128, MT, C], F32)
            st_eng = (nc.sync, nc.scalar, nc.gpsimd, nc.sync)
            for mt in range(MT):
                nc.vector.tensor_reduce(
                    S[:, mt:mt + 1], spart[:, mt * NPAIR:(mt + 1) * NPAIR],
                    axis=mybir.AxisListType.X, op=mybir.AluOpType.add,
                )
                nc.vector.reciprocal(rec[:, mt:mt + 1], S[:, mt:mt + 1])
                nc.vector.tensor_mul(ks[:, mt:mt + 1], S[:, mt:mt + 1],
                                     rec[:, mt:mt + 1])
                nc.vector.tensor_scalar_mul(
                    ot[:, mt, :], xt_sb[:, mt, :], ks[:, mt:mt + 1],
                )
                st_eng[mt].dma_start(out.ap()[:, mt], ot[:, mt, :])
            nc.scalar.dma_start(sdbg.ap(), S[:])

    nc.compile()
    return nc


def prep_inputs_bias(x, w_comp, b_comp, w_ker, b_ker):
    import ml_dtypes
    E4 = ml_dtypes.float8_e4m3
    xp = np.zeros((C, H + 2, W), np.float32)
    xp[:, 1:H + 1] = x
    wcT = _pmajor(np.ascontiguousarray(
        w_comp.reshape(CH, C).T).astype(ml_dtypes.bfloat16))
    bcr = np.ascontiguousarray(b_comp.reshape(CH, 1), dtype=np.float32)

    w9 = (w_ker.reshape(OC_TOTAL, CH, 9) * WSCALE).astype(E4)
    bk16 = (b_ker * WSCALE).astype(E4)

    in_maps = []
    for core in range(NCORES):
        g, h = core // 2, core % 2
        xfc = _pmajor(np.ascontiguousarray(
            xp[:, 16 * h:16 * h + HLOC].reshape(C, NLOC)
        ).astype(ml_dtypes.bfloat16))
        xtc = _pmajor(np.ascontiguousarray(
            x.reshape(C, H * W)[:, PIX * h:PIX * (h + 1)].T))
        a = w9[GCH * g:GCH * (g + 1)].reshape(NT, 512, CH, 9)
        wkc = np.empty((NT, 128, 2, 2, 512), E4)
        for kt, taps in enumerate((A_TAPS, B_TAPS)):
            for i in range(2):
                for hi in range(2):
                    wkc[:, 64 * hi:64 * (hi + 1), kt, i, :] = (
                        a[:, :, :, taps[i][hi]].transpose(0, 2, 1))
        t8 = a[:, :, :, 8].transpose(0, 2, 1)          # (NT, 64, 512)
        wktc = np.empty((NT, 65, 512), E4)
        wktc[:, 0:64] = t8
        wktc[:, 64] = bk16[GCH * g:GCH * (g + 1)].reshape(NT, 512)
        hmv = np.ones((CH, HLOC, W), np.float32)
        hmv[:, 0 if h == 0 else HLOC - 1] = 0.0
        im = {
            "xf": xfc,
            "xt": xtc,
            "wc": wcT,
            "bc": bcr,
            "wk": np.ascontiguousarray(wkc),
            "wkt": wktc,
            "hm": hmv.reshape(CH, NLOC).astype(E4),
        }
        in_maps.append(im)
    return in_maps
